# revision 13
# baseline (speedup 1.0000x reference)
"""CausalGNN forward on 8 Trainium2 NeuronCores (Bass/Tile).

Math (PyG-style GCN, 3 layers, BN training-mode, residuals):
    deg[v] = 1 + #{edges with dst=v};  dis = deg^-1/2
    per layer i:  h = x @ W_i;  agg[v] = sum_{e=(u,v)} dis_u dis_v h[u]
                  + dis_v^2 h[v]   (+ bias b_i, which BN cancels exactly)
                  y = BN(agg) (batch stats over all nodes), ReLU if i<2
                  x = y (i=0) or x + y (i>0)

Sharding: nodes (and the dst side of aggregation) are partitioned across 8
cores in contiguous ranges; edges live with their dst core, bucketed into
128-node chunks; self-edges ride the same path as ordinary edges. dis_u is
folded into the bf16 gather table (h' = dis*h); dis_v is folded into the
one-hot build (per-edge scalar multiply on the DVE).

Per layer, per core: h' (bf16) for own nodes -> AllGather table (Shared) ->
per group of chunks: a few big InstDMAGatherAnt batched gathers (int16
indices, table split in two halves to fit the int16 range; dead slots
gather row 0 and are masked by rel=-1) -> per tile a bf16 one-hot
[edge, dst]*dis_v on VectorE -> accumulate aggT[feature, dst] on TensorE in
PSUM -> PSUM->SBUF copy with fused BN sum + squared copy with fused BN
sumsq (ScalarE) -> AllReduce stats -> scale/shift (+ReLU) on ScalarE ->
residual on VectorE.

Host-side index bookkeeping: bucketing/sorting/padding edge lists, degrees
(bincount) -> dis tables, int16 wrapped gather indices, input/output
transposes, per-core slicing.
"""
import sys
sys.path.insert(0, "/opt/trn_rl_repo")

import numpy as np

import concourse.bass as bass
import concourse.tile as tile
from concourse import bacc, mybir

f32 = mybir.dt.float32
bf16 = mybir.dt.bfloat16
i16 = mybir.dt.int16
i32 = mybir.dt.int32

P = 128
CORES = 8
L = 3
EPS = 1e-5
HALF = 32768          # int16 index range limit for dma_gather
GG = 4                # chunks per gather group


# ---------------------------------------------------------------- host prep

def _prep(x, edge_index):
    """Bucket edges by (core, chunk, table-half), append self-edges, pad to
    128-slot tiles, and build wrapped int16 gather indices.

    Slot j of a section maps to (partition j%128, tile sec_base + j//128).
    Dead slots gather table row 0 (valid) and carry rel=-1, disv=0 so they
    contribute nothing.
    """
    N, D = x.shape
    E = edge_index.shape[1]
    n_own = (N + CORES - 1) // CORES
    n_pad = ((n_own + P - 1) // P) * P
    n_chunks = n_pad // P

    src = edge_index[0].astype(np.int64)
    dst = edge_index[1].astype(np.int64)

    deg = (np.bincount(dst, minlength=N) + 1.0).astype(np.float64)
    dis = (1.0 / np.sqrt(deg)).astype(np.float32)

    def table_row(n):
        c = n // n_own
        return c * n_pad + (n - c * n_own)

    # edges + self-edges as one list of (core, chunk, rel, row, disv)
    nodes = np.arange(N, dtype=np.int64)
    s_core = nodes // n_own
    s_local = nodes - s_core * n_own
    a_core = np.concatenate([dst // n_own, s_core])
    a_local = np.concatenate([dst - (dst // n_own) * n_own, s_local])
    a_row = np.concatenate([table_row(src), s_core * n_pad + s_local])
    a_disv = dis[np.concatenate([dst, nodes])]
    a_chunk = a_local // P
    a_rel = (a_local % P).astype(np.float32)
    a_hi = (a_row >= HALF).astype(np.int64)

    # shared (SPMD) per-chunk lo/hi tile counts
    nlo = np.zeros((CORES, n_chunks), np.int64)
    nhi = np.zeros((CORES, n_chunks), np.int64)
    np.add.at(nlo, (a_core[a_hi == 0], a_chunk[a_hi == 0]), 1)
    np.add.at(nhi, (a_core[a_hi == 1], a_chunk[a_hi == 1]), 1)
    k_lo = np.maximum((nlo.max(axis=0) + P - 1) // P, 1).astype(np.int64)
    k_hi = np.maximum((nhi.max(axis=0) + P - 1) // P, 1).astype(np.int64)

    # groups of GG chunks; per-group tile layout:
    #   [lo c0][lo c1]...[hi c0][hi c1]...
    groups = []          # {chunks, base, n_tiles, gathers:[(off,nt,colb,hi)]}
    chunk_tiles = [None] * n_chunks   # global tile indices per chunk
    sec_of_chunk = {}
    base = 0
    for g0 in range(0, n_chunks, GG):
        chs = list(range(g0, min(g0 + GG, n_chunks)))
        gathers = []
        off = 0
        for hi in (0, 1):
            for c in chs:
                nt = int((k_lo if hi == 0 else k_hi)[c])
                gathers.append((off, nt, (base + off) * 8, hi))
                sec_of_chunk[(c, hi)] = base + off
                off += nt
        groups.append({"chunks": chs, "base": base, "n_tiles": off,
                       "gathers": gathers})
        for c in chs:
            lo_t = list(range(sec_of_chunk[(c, 0)],
                              sec_of_chunk[(c, 0)] + int(k_lo[c])))
            hi_t = list(range(sec_of_chunk[(c, 1)],
                              sec_of_chunk[(c, 1)] + int(k_hi[c])))
            chunk_tiles[c] = lo_t + hi_t
        base += off
    NT = base

    rel_arr = np.full((CORES, P, NT), -1.0, np.float32)
    disv_arr = np.zeros((CORES, P, NT), np.float32)
    widx = np.zeros((CORES, P, NT * 8), np.int16)
    src_arr = np.zeros((CORES, P, NT), np.int32)   # per-tile fallback path

    # fill slots: sort by (core, chunk, hi); positions within bucket
    okey = np.lexsort((a_hi, a_chunk, a_core))
    c_o, ch_o, hi_o = a_core[okey], a_chunk[okey], a_hi[okey]
    rel_o, row_o, disv_o = a_rel[okey], a_row[okey], a_disv[okey]
    bucket = (c_o * n_chunks + ch_o) * 2 + hi_o
    nb = CORES * n_chunks * 2
    starts = np.searchsorted(bucket, np.arange(nb))
    pos = np.arange(len(bucket)) - starts[bucket]
    sec_base = np.zeros(nb, np.int64)
    for c in range(CORES):
        for ch in range(n_chunks):
            for hi in (0, 1):
                sec_base[(c * n_chunks + ch) * 2 + hi] = sec_of_chunk[(ch, hi)]
    t_glob = sec_base[bucket] + pos // P
    p_idx = pos % P
    rel_arr[c_o, p_idx, t_glob] = rel_o
    disv_arr[c_o, p_idx, t_glob] = disv_o
    src_arr[c_o, p_idx, t_glob] = row_o
    val = np.where(hi_o == 1, row_o - HALF, row_o).astype(np.int16)
    # wrapped: global flat i = t_glob*128 + p -> [i%16, i//16]
    i_flat = t_glob * P + p_idx
    widx[c_o, i_flat % 16, i_flat // 16] = val
    for rblk in range(1, 8):
        widx[:, rblk * 16:(rblk + 1) * 16, :] = widx[:, 0:16, :]

    xT = np.zeros((CORES, D, n_pad), np.float32)
    disc = np.ones((CORES, P, n_chunks), np.float32)
    for c in range(CORES):
        lo, hi2 = c * n_own, min((c + 1) * n_own, N)
        xT[c, :, :hi2 - lo] = x[lo:hi2].T
        dcol = np.ones(n_pad, np.float32)
        dcol[:hi2 - lo] = dis[lo:hi2]
        disc[c] = dcol.reshape(n_chunks, P).T
    return {"xT": xT, "widx": widx, "src": src_arr, "rel": rel_arr,
            "disv": disv_arr, "disc": disc, "groups": groups,
            "chunk_tiles": chunk_tiles, "n_own": n_own, "n_pad": n_pad,
            "n_chunks": n_chunks, "NT": NT}


# ------------------------------------------------------------- device build

def _build(D, n_pad, n_chunks, NT, groups, chunk_tiles, n_real_last, N_total):
    """Build the SPMD Bass program (same for all cores)."""
    import os
    gmode = os.environ.get("KERNEL_GATHER", "ant")   # ant | tile
    shared = os.environ.get("KERNEL_SHARED", "1") == "1"
    nc = bacc.Bacc("TRN2", target_bir_lowering=False, debug=False,
                   num_devices=CORES)
    TBL = CORES * n_pad
    gmax = max(g["n_tiles"] for g in groups)

    xT_in = nc.dram_tensor("xT_in", [D, n_pad], f32, kind="ExternalInput")
    widx_in = nc.dram_tensor("widx_in", [P, NT * 8], i16,
                             kind="ExternalInput")
    src_in = (nc.dram_tensor("src_in", [P, NT], i32, kind="ExternalInput")
              if gmode == "tile" else None)
    rel_in = nc.dram_tensor("rel_in", [P, NT], f32, kind="ExternalInput")
    disv_in = nc.dram_tensor("disv_in", [P, NT], f32, kind="ExternalInput")
    disc_in = nc.dram_tensor("disc_in", [P, n_chunks], f32,
                             kind="ExternalInput")
    Ws_in = nc.dram_tensor("Ws_in", [L * D, D], f32, kind="ExternalInput")
    gb_in = nc.dram_tensor("gb_in", [D, 2 * L], f32, kind="ExternalInput")
    out_ext = nc.dram_tensor("out", [D, n_pad], f32, kind="ExternalOutput")

    h_own = nc.dram_tensor("h_own", [n_pad, D], bf16)
    h_tbl = nc.dram_tensor("h_tbl", [TBL, D], bf16,
                           addr_space="Shared" if shared else "Local")
    st_in = nc.dram_tensor("st_in", [P, 2], f32)
    st_out = nc.dram_tensor("st_out", [CORES, P, 2], f32)

    RG = [list(range(CORES))]
    AOP = mybir.AluOpType

    with tile.TileContext(nc) as tc:
        with tc.tile_pool(name="big", bufs=1) as big, \
             tc.tile_pool(name="sm", bufs=1) as sm, \
             tc.tile_pool(name="gat", bufs=3) as gat, \
             tc.tile_pool(name="oh", bufs=8) as ohp, \
             tc.tile_pool(name="work", bufs=4) as wk, \
             tc.tile_pool(name="ps", bufs=2, space="PSUM") as ps, \
             tc.tile_pool(name="psa", bufs=4, space="PSUM") as psa:

            # ---------------- persistent SBUF state
            xT = big.tile([D, n_pad], f32)
            nc.sync.dma_start(out=xT[:], in_=xT_in[:, :])
            widx_sb = big.tile([P, NT * 8], i16)
            nc.sync.dma_start(out=widx_sb[:], in_=widx_in[:, :])
            if gmode == "tile":
                src_sb = big.tile([P, NT], i32)
                nc.sync.dma_start(out=src_sb[:], in_=src_in[:, :])
            rel_sb = big.tile([P, NT], f32)
            nc.sync.dma_start(out=rel_sb[:], in_=rel_in[:, :])
            disv_sb = big.tile([P, NT], f32)
            nc.sync.dma_start(out=disv_sb[:], in_=disv_in[:, :])
            disc_sb = sm.tile([P, n_chunks], f32)
            nc.sync.dma_start(out=disc_sb[:], in_=disc_in[:, :])
            Ws_sb = sm.tile([D, L * D], f32)
            for i in range(L):
                nc.sync.dma_start(out=Ws_sb[:, i * D:(i + 1) * D],
                                  in_=Ws_in[i * D:(i + 1) * D, :])
            gb_sb = sm.tile([D, 2 * L], f32)
            nc.sync.dma_start(out=gb_sb[:], in_=gb_in[:, :])

            iota_i = sm.tile([P, P], i32)
            nc.gpsimd.iota(iota_i[:], pattern=[[1, P]], base=0,
                           channel_multiplier=0)
            iota_bf = sm.tile([P, P], bf16)
            nc.vector.tensor_copy(iota_bf[:], iota_i[:])

            agg = big.tile([D, n_pad], f32)
            n_own_cols = (n_chunks - 1) * P + n_real_last
            if n_own_cols < n_pad:
                nc.vector.memset(agg[:, n_own_cols:], 0.0)
            slots = sm.tile([P, 2 * n_chunks], f32)
            stat = sm.tile([P, 8], f32)
            h_all = big.tile([P, n_chunks * D], bf16)  # staged h' rows

            # ---------------- layers
            inv_n = 1.0 / float(N_total)
            for i in range(L):
                # h' = dis * (x @ W_i) in bf16, staged then one DMA to DRAM
                for ch in range(n_chunks):
                    hps = ps.tile([P, D], f32, space="PSUM", tag="h")
                    nc.tensor.matmul(out=hps[:],
                                     lhsT=xT[:, ch * P:(ch + 1) * P],
                                     rhs=Ws_sb[:, i * D:(i + 1) * D],
                                     start=True, stop=True)
                    nc.scalar.mul(out=h_all[:, ch * D:(ch + 1) * D],
                                  in_=hps[:], mul=disc_sb[:, ch:ch + 1])
                nc.sync.dma_start(
                    out=h_own[:, :].rearrange("(c p) d -> p c d", p=P),
                    in_=h_all[:])
                nc.gpsimd.collective_compute(
                    "AllGather", AOP.bypass, replica_groups=RG,
                    ins=[h_own[:, :]], outs=[h_tbl[:, :]])

                # edge phase: batched gathers per group, one-hot matmuls
                for grp in groups:
                    g = gat.tile([P, gmax * D], bf16, tag="g")
                    if gmode == "ant":
                        for (off, nt, colb, hi) in grp["gathers"]:
                            nc.gpsimd.dma_gather(
                                out_ap=g[:, off * D:(off + nt) * D].rearrange(
                                    "p (t e) -> p t e", e=D),
                                in_ap=(h_tbl[HALF:, :] if hi
                                       else h_tbl[:, :]),
                                idxs_ap=widx_sb[:, colb:colb + nt * 8],
                                num_idxs=nt * P, num_idxs_reg=nt * P,
                                elem_size=D)
                    else:
                        for t in range(grp["base"],
                                       grp["base"] + grp["n_tiles"]):
                            off = t - grp["base"]
                            nc.gpsimd.indirect_dma_start(
                                out=g[:, off * D:(off + 1) * D],
                                out_offset=None,
                                in_=h_tbl[:, :],
                                in_offset=bass.IndirectOffsetOnAxis(
                                    ap=src_sb[:, t:t + 1], axis=0))
                    for ch in grp["chunks"]:
                        tl = chunk_tiles[ch]
                        aps = psa.tile([D, P], f32, space="PSUM", tag="agg")
                        for j, t in enumerate(tl):
                            goff = t - grp["base"]
                            oht = ohp.tile([P, P], bf16, tag="oh")
                            nc.vector.tensor_scalar(
                                out=oht[:], in0=iota_bf[:],
                                scalar1=rel_sb[:, t:t + 1],
                                scalar2=disv_sb[:, t:t + 1],
                                op0=AOP.is_equal, op1=AOP.mult)
                            nc.tensor.matmul(
                                out=aps[:],
                                lhsT=g[:, goff * D:(goff + 1) * D],
                                rhs=oht[:],
                                start=(j == 0), stop=(j == len(tl) - 1))
                        # PSUM -> SBUF with fused BN sums on the Scalar engine
                        w = P if ch < n_chunks - 1 else n_real_last
                        nc.scalar.activation(
                            out=agg[:, ch * P:ch * P + w], in_=aps[:, 0:w],
                            func=mybir.ActivationFunctionType.Copy,
                            accum_out=slots[:, ch:ch + 1])
                        sq = wk.tile([D, P], f32, tag="sq")
                        nc.scalar.activation(
                            out=sq[:, 0:w], in_=aps[:, 0:w],
                            func=mybir.ActivationFunctionType.Square,
                            accum_out=slots[:,
                                            n_chunks + ch:n_chunks + ch + 1])

                # stats: reduce chunk slots, AllGather + local cross-core sum
                nc.vector.tensor_reduce(
                    out=stat[:, 0:1], in_=slots[:, 0:n_chunks],
                    axis=mybir.AxisListType.X, op=AOP.add)
                nc.vector.tensor_reduce(
                    out=stat[:, 1:2], in_=slots[:, n_chunks:2 * n_chunks],
                    axis=mybir.AxisListType.X, op=AOP.add)
                sin = wk.tile([P, 2], f32, tag="stin")
                nc.vector.tensor_copy(sin[:], stat[:, 0:2])
                nc.sync.dma_start(out=st_in[:, :], in_=sin[:])
                nc.gpsimd.collective_compute(
                    "AllGather", AOP.bypass, replica_groups=RG,
                    ins=[st_in[:, :]], outs=[st_out[:, :, :]])
                # read back as [p, (k, c)]: cols 0:8 sums, 8:16 sumsqs
                sall = wk.tile([P, 2 * CORES], f32, tag="sall")
                nc.sync.dma_start(
                    out=sall[:],
                    in_=st_out[:, :, :].transpose([1, 2, 0]))
                sout = wk.tile([P, 2], f32, tag="stout")
                nc.vector.tensor_reduce(
                    out=sout[:, 0:1], in_=sall[:, 0:CORES],
                    axis=mybir.AxisListType.X, op=AOP.add)
                nc.vector.tensor_reduce(
                    out=sout[:, 1:2], in_=sall[:, CORES:2 * CORES],
                    axis=mybir.AxisListType.X, op=AOP.add)
                # mean, var, scale = gamma*rsqrt(var+eps), shift = beta-sc*mean
                nc.vector.tensor_scalar(out=stat[:, 2:3], in0=sout[:, 0:1],
                                        scalar1=inv_n, scalar2=None,
                                        op0=AOP.mult)
                nc.vector.tensor_scalar(out=stat[:, 3:4], in0=sout[:, 1:2],
                                        scalar1=inv_n, scalar2=None,
                                        op0=AOP.mult)
                nc.vector.tensor_tensor(out=stat[:, 4:5], in0=stat[:, 2:3],
                                        in1=stat[:, 2:3], op=AOP.mult)
                nc.vector.tensor_tensor(out=stat[:, 4:5], in0=stat[:, 3:4],
                                        in1=stat[:, 4:5], op=AOP.subtract)
                nc.vector.tensor_scalar(out=stat[:, 4:5], in0=stat[:, 4:5],
                                        scalar1=float(EPS), scalar2=None,
                                        op0=AOP.add)
                nc.vector.reciprocal(stat[:, 5:6], stat[:, 4:5])
                nc.scalar.sqrt(stat[:, 6:7], stat[:, 5:6])
                nc.vector.tensor_tensor(out=stat[:, 6:7],
                                        in0=gb_sb[:, 2 * i:2 * i + 1],
                                        in1=stat[:, 6:7], op=AOP.mult)
                nc.vector.tensor_tensor(out=stat[:, 7:8], in0=stat[:, 6:7],
                                        in1=stat[:, 2:3], op=AOP.mult)
                nc.vector.tensor_tensor(out=stat[:, 7:8],
                                        in0=gb_sb[:, 2 * i + 1:2 * i + 2],
                                        in1=stat[:, 7:8], op=AOP.subtract)

                # y = func(scale*agg + shift); x = y or x + y
                # (single big ops: everything is gated on the stats anyway)
                func = (mybir.ActivationFunctionType.Relu if i < L - 1
                        else mybir.ActivationFunctionType.Identity)
                if i == 0:
                    nc.scalar.activation(out=xT[:], in_=agg[:],
                                         func=func, bias=stat[:, 7:8],
                                         scale=stat[:, 6:7])
                else:
                    nc.scalar.activation(out=agg[:], in_=agg[:],
                                         func=func, bias=stat[:, 7:8],
                                         scale=stat[:, 6:7])
                    nc.vector.tensor_tensor(out=xT[:], in0=xT[:],
                                            in1=agg[:], op=AOP.add)

            nc.sync.dma_start(out=out_ext[:, :], in_=xT[:])
    nc.compile()
    return nc


# ------------------------------------------------------------------ runner

class _Runner:
    """Persistent-jit PJRT runner (run_bass_via_pjrt, callable repeatedly)."""

    def __init__(self, nc, n_cores):
        import jax
        from jax.experimental.shard_map import shard_map
        from jax.sharding import Mesh, PartitionSpec
        from concourse import bass2jax
        self.jax = jax
        bass2jax.install_neuronx_cc_hook()
        in_names, out_names, out_avals, zero_outs = [], [], [], []
        partition_name = (nc.partition_id_tensor.name
                          if nc.partition_id_tensor else None)
        for alloc in nc.m.functions[0].allocations:
            if not isinstance(alloc, mybir.MemoryLocationSet):
                continue
            name = alloc.memorylocations[0].name
            if alloc.kind == "ExternalInput":
                if name != partition_name:
                    in_names.append(name)
            elif alloc.kind == "ExternalOutput":
                out_names.append(name)
                shape = tuple(alloc.tensor_shape)
                dtype = mybir.dt.np(alloc.dtype)
                out_avals.append(jax.core.ShapedArray(shape, dtype))
                zero_outs.append(np.zeros(shape, dtype))
        self.in_names, self.out_names = in_names, out_names
        self.out_avals, self.zero_outs = out_avals, zero_outs
        n_params, n_outs = len(in_names), len(out_avals)
        all_in = list(in_names) + list(out_names)
        if partition_name is not None:
            all_in.append(partition_name)
        from concourse.bass2jax import _bass_exec_p, partition_id_tensor

        def _body(*args):
            operands = list(args)
            if partition_name is not None:
                operands.append(partition_id_tensor())
            outs = _bass_exec_p.bind(
                *operands, out_avals=tuple(out_avals),
                in_names=tuple(all_in), out_names=tuple(out_names),
                lowering_input_output_aliases=(),
                sim_require_finite=False, sim_require_nnan=False, nc=nc)
            return tuple(outs)

        devices = jax.devices()[:n_cores]
        self.n_cores = n_cores
        self.mesh = Mesh(np.asarray(devices), ("core",))
        in_specs = (PartitionSpec("core"),) * (n_params + n_outs)
        out_specs = (PartitionSpec("core"),) * len(out_names)
        self.fn = jax.jit(
            shard_map(_body, mesh=self.mesh, in_specs=in_specs,
                      out_specs=out_specs, check_rep=False),
            keep_unused=True)
        self.dev_in = None

    def put(self, in_maps):
        from jax.sharding import NamedSharding, PartitionSpec
        sh = NamedSharding(self.mesh, PartitionSpec("core"))
        n = self.n_cores
        concat_in = [
            np.concatenate([np.asarray(in_maps[c][name]) for c in range(n)],
                           axis=0)
            for name in self.in_names]
        concat_zeros = [np.zeros((n * z.shape[0], *z.shape[1:]), z.dtype)
                        for z in self.zero_outs]
        self.dev_in = [self.jax.device_put(a, sh)
                       for a in concat_in + concat_zeros]
        self.jax.block_until_ready(self.dev_in)

    def __call__(self, fetch=("out",)):
        out = self.fn(*self.dev_in)
        self.jax.block_until_ready(out)
        n = self.n_cores
        return [
            {name: np.asarray(out[i]).reshape(n, *self.out_avals[i].shape)[c]
             for i, name in enumerate(self.out_names) if name in fetch}
            for c in range(n)]


_CACHE = {}


def _build_from_prep(prep, N, D):
    n_real_last = prep["n_own"] - (prep["n_chunks"] - 1) * P
    return _build(D, prep["n_pad"], prep["n_chunks"], prep["NT"],
                  prep["groups"], prep["chunk_tiles"], n_real_last, N)


def _get_runner(prep, N, D):
    key = (N, D, prep["NT"])
    if key in _CACHE:
        return _CACHE[key]
    nc = _build_from_prep(prep, N, D)
    r = _Runner(nc, CORES)
    _CACHE[key] = r
    return r


def _make_in_maps(prep, x, Ws, bs, gammas, betas):
    D = x.shape[1]
    Ws_flat = np.asarray(Ws, np.float32).reshape(L * D, D)
    gb = np.zeros((D, 2 * L), np.float32)
    for i in range(L):
        gb[:, 2 * i] = gammas[i]
        gb[:, 2 * i + 1] = betas[i]
    return [{"xT_in": prep["xT"][c], "widx_in": prep["widx"][c],
             "src_in": prep["src"][c], "rel_in": prep["rel"][c],
             "disv_in": prep["disv"][c], "disc_in": prep["disc"][c],
             "Ws_in": Ws_flat, "gb_in": gb} for c in range(CORES)]


def _assemble_out(prep, res, N, D):
    n_own = prep["n_own"]
    out = np.empty((N, D), np.float32)
    for c in range(CORES):
        lo, hi = c * n_own, min((c + 1) * n_own, N)
        out[lo:hi] = res[c]["out"][:, :hi - lo].T
    return out


def kernel(x, edge_index, Ws, bs, gammas, betas):
    x = np.asarray(x, np.float32)
    edge_index = np.asarray(edge_index, np.int32)
    Ws = np.asarray(Ws, np.float32)
    gammas = np.asarray(gammas, np.float32)
    betas = np.asarray(betas, np.float32)
    N, D = x.shape

    prep = _prep(x, edge_index)
    r = _get_runner(prep, N, D)
    in_maps = _make_in_maps(prep, x, Ws, bs, gammas, betas)
    r.put(in_maps)
    res = r()
    return _assemble_out(prep, res, N, D)


# revision 14
# speedup vs baseline: 1.1211x; 1.1211x over previous
"""CausalGNN forward on 8 Trainium2 NeuronCores (Bass/Tile).

Math (PyG-style GCN, 3 layers, BN training-mode, residuals):
    deg[v] = 1 + #{edges with dst=v};  dis = deg^-1/2
    per layer i:  h = x @ W_i;  agg[v] = sum_{e=(u,v)} dis_u dis_v h[u]
                  + dis_v^2 h[v]   (+ bias b_i, which BN cancels exactly)
                  y = BN(agg) (batch stats over all nodes), ReLU if i<2
                  x = y (i=0) or x + y (i>0)

Sharding: nodes (and the dst side of aggregation) are partitioned across 8
cores in contiguous ranges; edges live with their dst core, bucketed into
128-node chunks. dis_u is folded into the bf16 gather table (h' = dis*h);
dis_v is folded into the one-hot build (per-edge scalar multiply on DVE).
The self-loop term dis_v^2 h[v] is NOT in the edge stream: it is a direct
W_i^T @ (dis^2 * x) matmul accumulated into the same PSUM tile as the edge
messages (first accumulation step of each chunk).

Per layer, per core: h' (bf16) for own nodes (staged in SBUF, one DMA) ->
AllGather table (Shared) -> per chunk: per-tile indirect gathers (128 rows
each; the HW DGE supports one index per partition per instruction) -> per
tile a bf16 one-hot [edge, dst]*dis_v on VectorE -> accumulate
aggT[feature, dst] on TensorE in PSUM -> PSUM->SBUF copy with fused BN sum
+ squared copy with fused BN sumsq (ScalarE) -> stats AllGather + local
cross-core reduce -> one big scale/shift (+ReLU) on ScalarE -> residual on
VectorE.

Host-side index bookkeeping: bucketing/sorting/padding edge lists, degrees
(bincount) -> dis tables, input/output transposes, per-core slicing.
"""
import sys
sys.path.insert(0, "/opt/trn_rl_repo")

import numpy as np

import concourse.bass as bass
import concourse.tile as tile
from concourse import bacc, mybir

f32 = mybir.dt.float32
bf16 = mybir.dt.bfloat16
i32 = mybir.dt.int32

P = 128
CORES = 8
L = 3
EPS = 1e-5


# ---------------------------------------------------------------- host prep

def _prep(x, edge_index):
    """Bucket edges by (core, chunk), pad to 128-slot tiles.

    Host computes degrees -> dis = deg^-1/2 (index bookkeeping:
    deg = bincount(dst) + 1). Pad slots carry rel=-1, disv=0 and gather
    table row 0, so they contribute nothing.
    """
    N, D = x.shape
    E = edge_index.shape[1]
    n_own = (N + CORES - 1) // CORES
    n_pad = ((n_own + P - 1) // P) * P
    n_chunks = n_pad // P

    src = edge_index[0].astype(np.int64)
    dst = edge_index[1].astype(np.int64)

    deg = (np.bincount(dst, minlength=N) + 1.0).astype(np.float64)
    dis = (1.0 / np.sqrt(deg)).astype(np.float32)

    def table_row(n):
        c = n // n_own
        return c * n_pad + (n - c * n_own)

    core_of = dst // n_own
    local = dst - core_of * n_own
    chunk_of = local // P
    dst_rel = (local % P).astype(np.float32)
    rows_e = table_row(src)
    disv_e = dis[dst]

    counts = np.zeros((CORES, n_chunks), np.int64)
    np.add.at(counts, (core_of, chunk_of), 1)
    tiles_per_chunk = np.maximum(
        (counts.max(axis=0) + P - 1) // P, 1).astype(np.int64)
    tile_base = np.concatenate([[0], np.cumsum(tiles_per_chunk)])
    NT = int(tile_base[-1])

    src_arr = np.zeros((CORES, P, NT), np.int32)
    rel_arr = np.full((CORES, P, NT), -1.0, np.float32)
    disv_arr = np.zeros((CORES, P, NT), np.float32)

    order = np.lexsort((chunk_of, core_of))
    c_o, ch_o = core_of[order], chunk_of[order]
    rel_o, row_o, disv_o = dst_rel[order], rows_e[order], disv_e[order]
    bucket = c_o * n_chunks + ch_o
    starts = np.searchsorted(bucket, np.arange(CORES * n_chunks))
    pos = np.arange(E) - starts[bucket]
    t_idx = tile_base[ch_o] + pos // P
    p_idx = pos % P
    src_arr[c_o, p_idx, t_idx] = row_o
    rel_arr[c_o, p_idx, t_idx] = rel_o
    disv_arr[c_o, p_idx, t_idx] = disv_o

    xT = np.zeros((CORES, D, n_pad), np.float32)
    disc = np.ones((CORES, P, n_chunks), np.float32)
    dis2b = np.zeros((CORES, D, n_pad), np.float32)
    for c in range(CORES):
        lo, hi = c * n_own, min((c + 1) * n_own, N)
        xT[c, :, :hi - lo] = x[lo:hi].T
        dcol = np.ones(n_pad, np.float32)
        dcol[:hi - lo] = dis[lo:hi]
        disc[c] = dcol.reshape(n_chunks, P).T
        d2 = np.zeros(n_pad, np.float32)
        d2[:hi - lo] = dis[lo:hi] ** 2
        dis2b[c, :, :] = d2[None, :]
    return {"xT": xT, "src": src_arr, "rel": rel_arr, "disv": disv_arr,
            "disc": disc, "dis2b": dis2b,
            "tpc": tiles_per_chunk.astype(int), "n_own": n_own,
            "n_pad": n_pad, "n_chunks": n_chunks, "NT": NT}


# ------------------------------------------------------------- device build

def _build(D, n_pad, n_chunks, NT, tiles_per_chunk, n_real_last, N_total):
    """Build the SPMD Bass program (same for all cores)."""
    nc = bacc.Bacc("TRN2", target_bir_lowering=False, debug=False,
                   num_devices=CORES)
    TBL = CORES * n_pad
    kmax = int(max(tiles_per_chunk))

    xT_in = nc.dram_tensor("xT_in", [D, n_pad], f32, kind="ExternalInput")
    src_in = nc.dram_tensor("src_in", [P, NT], i32, kind="ExternalInput")
    rel_in = nc.dram_tensor("rel_in", [P, NT], f32, kind="ExternalInput")
    disv_in = nc.dram_tensor("disv_in", [P, NT], f32, kind="ExternalInput")
    disc_in = nc.dram_tensor("disc_in", [P, n_chunks], f32,
                             kind="ExternalInput")
    dis2_in = nc.dram_tensor("dis2_in", [D, n_pad], f32,
                             kind="ExternalInput")
    Ws_in = nc.dram_tensor("Ws_in", [L * D, D], f32, kind="ExternalInput")
    gb_in = nc.dram_tensor("gb_in", [D, 2 * L], f32, kind="ExternalInput")
    out_ext = nc.dram_tensor("out", [D, n_pad], f32, kind="ExternalOutput")

    h_own = nc.dram_tensor("h_own", [n_pad, D], bf16)
    h_tbl = nc.dram_tensor("h_tbl", [TBL, D], bf16, addr_space="Shared")
    st_in = nc.dram_tensor("st_in", [P, 2], f32)
    st_out = nc.dram_tensor("st_out", [CORES, P, 2], f32)

    RG = [list(range(CORES))]
    AOP = mybir.AluOpType

    with tile.TileContext(nc) as tc:
        with tc.tile_pool(name="big", bufs=1) as big, \
             tc.tile_pool(name="sm", bufs=1) as sm, \
             tc.tile_pool(name="gat", bufs=4) as gat, \
             tc.tile_pool(name="oh", bufs=8) as ohp, \
             tc.tile_pool(name="work", bufs=4) as wk, \
             tc.tile_pool(name="ps", bufs=2, space="PSUM") as ps, \
             tc.tile_pool(name="psa", bufs=4, space="PSUM") as psa:

            # ---------------- persistent SBUF state
            xT = big.tile([D, n_pad], f32)
            nc.sync.dma_start(out=xT[:], in_=xT_in[:, :])
            src_sb = big.tile([P, NT], i32)
            nc.sync.dma_start(out=src_sb[:], in_=src_in[:, :])
            rel_sb = big.tile([P, NT], f32)
            nc.sync.dma_start(out=rel_sb[:], in_=rel_in[:, :])
            disv_sb = big.tile([P, NT], f32)
            nc.sync.dma_start(out=disv_sb[:], in_=disv_in[:, :])
            disc_sb = sm.tile([P, n_chunks], f32)
            nc.sync.dma_start(out=disc_sb[:], in_=disc_in[:, :])
            dis2_sb = big.tile([D, n_pad], f32)
            nc.sync.dma_start(out=dis2_sb[:], in_=dis2_in[:, :])
            Ws_sb = sm.tile([D, L * D], f32)
            for i in range(L):
                nc.sync.dma_start(out=Ws_sb[:, i * D:(i + 1) * D],
                                  in_=Ws_in[i * D:(i + 1) * D, :])
            gb_sb = sm.tile([D, 2 * L], f32)
            nc.sync.dma_start(out=gb_sb[:], in_=gb_in[:, :])

            iota_i = sm.tile([P, P], i32)
            nc.gpsimd.iota(iota_i[:], pattern=[[1, P]], base=0,
                           channel_multiplier=0)
            iota_bf = sm.tile([P, P], bf16)
            nc.vector.tensor_copy(iota_bf[:], iota_i[:])

            agg = big.tile([D, n_pad], f32)
            n_own_cols = (n_chunks - 1) * P + n_real_last
            if n_own_cols < n_pad:
                nc.vector.memset(agg[:, n_own_cols:], 0.0)
            slots = sm.tile([P, 2 * n_chunks], f32)
            stat = sm.tile([P, 8], f32)
            h_all = big.tile([P, n_chunks * D], bf16)  # staged h' rows
            xs = big.tile([D, n_pad], f32)             # dis^2 * x

            tb = np.concatenate([[0], np.cumsum(tiles_per_chunk)]).astype(int)

            # ---------------- layers
            inv_n = 1.0 / float(N_total)
            for i in range(L):
                # h' = dis * (x @ W_i) in bf16, staged then one DMA to DRAM
                for ch in range(n_chunks):
                    hps = ps.tile([P, D], f32, space="PSUM", tag="h")
                    nc.tensor.matmul(out=hps[:],
                                     lhsT=xT[:, ch * P:(ch + 1) * P],
                                     rhs=Ws_sb[:, i * D:(i + 1) * D],
                                     start=True, stop=True)
                    nc.scalar.mul(out=h_all[:, ch * D:(ch + 1) * D],
                                  in_=hps[:], mul=disc_sb[:, ch:ch + 1])
                nc.sync.dma_start(
                    out=h_own[:, :].rearrange("(c p) d -> p c d", p=P),
                    in_=h_all[:])
                nc.gpsimd.collective_compute(
                    "AllGather", AOP.bypass, replica_groups=RG,
                    ins=[h_own[:, :]], outs=[h_tbl[:, :]])
                # xs = dis^2 * x for the self-loop term (one big DVE op)
                nc.vector.tensor_tensor(out=xs[:], in0=xT[:], in1=dis2_sb[:],
                                        op=AOP.mult)

                # edge phase: per-tile gathers, one-hot matmuls per chunk
                for ch in range(n_chunks):
                    k = int(tb[ch + 1] - tb[ch])
                    g = gat.tile([P, kmax * D], bf16, tag="g")
                    for j in range(k):
                        t = int(tb[ch]) + j
                        nc.gpsimd.indirect_dma_start(
                            out=g[:, j * D:(j + 1) * D], out_offset=None,
                            in_=h_tbl[:, :],
                            in_offset=bass.IndirectOffsetOnAxis(
                                ap=src_sb[:, t:t + 1], axis=0))
                    aps = psa.tile([D, P], f32, space="PSUM", tag="agg")
                    # self-loop term: aps = W_i^T @ (dis^2 * x)[:, chunk]
                    nc.tensor.matmul(out=aps[:],
                                     lhsT=Ws_sb[:, i * D:(i + 1) * D],
                                     rhs=xs[:, ch * P:(ch + 1) * P],
                                     start=True, stop=False)
                    for j in range(k):
                        t = int(tb[ch]) + j
                        oht = ohp.tile([P, P], bf16, tag="oh")
                        nc.vector.tensor_scalar(
                            out=oht[:], in0=iota_bf[:],
                            scalar1=rel_sb[:, t:t + 1],
                            scalar2=disv_sb[:, t:t + 1],
                            op0=AOP.is_equal, op1=AOP.mult)
                        nc.tensor.matmul(out=aps[:],
                                         lhsT=g[:, j * D:(j + 1) * D],
                                         rhs=oht[:],
                                         start=False, stop=(j == k - 1))
                    # PSUM -> SBUF with fused BN sums on the Scalar engine
                    w = P if ch < n_chunks - 1 else n_real_last
                    nc.scalar.activation(
                        out=agg[:, ch * P:ch * P + w], in_=aps[:, 0:w],
                        func=mybir.ActivationFunctionType.Copy,
                        accum_out=slots[:, ch:ch + 1])
                    sq = wk.tile([D, P], f32, tag="sq")
                    nc.scalar.activation(
                        out=sq[:, 0:w], in_=aps[:, 0:w],
                        func=mybir.ActivationFunctionType.Square,
                        accum_out=slots[:, n_chunks + ch:n_chunks + ch + 1])

                # stats: reduce chunk slots, AllGather + local cross-core sum
                nc.vector.tensor_reduce(
                    out=stat[:, 0:1], in_=slots[:, 0:n_chunks],
                    axis=mybir.AxisListType.X, op=AOP.add)
                nc.vector.tensor_reduce(
                    out=stat[:, 1:2], in_=slots[:, n_chunks:2 * n_chunks],
                    axis=mybir.AxisListType.X, op=AOP.add)
                sin = wk.tile([P, 2], f32, tag="stin")
                nc.vector.tensor_copy(sin[:], stat[:, 0:2])
                nc.sync.dma_start(out=st_in[:, :], in_=sin[:])
                nc.gpsimd.collective_compute(
                    "AllGather", AOP.bypass, replica_groups=RG,
                    ins=[st_in[:, :]], outs=[st_out[:, :, :]])
                sall = wk.tile([P, 2 * CORES], f32, tag="sall")
                nc.sync.dma_start(
                    out=sall[:],
                    in_=st_out[:, :, :].transpose([1, 2, 0]))
                sout = wk.tile([P, 2], f32, tag="stout")
                nc.vector.tensor_reduce(
                    out=sout[:, 0:1], in_=sall[:, 0:CORES],
                    axis=mybir.AxisListType.X, op=AOP.add)
                nc.vector.tensor_reduce(
                    out=sout[:, 1:2], in_=sall[:, CORES:2 * CORES],
                    axis=mybir.AxisListType.X, op=AOP.add)
                # mean, var, scale = gamma*rsqrt(var+eps), shift = beta-sc*mean
                nc.vector.tensor_scalar(out=stat[:, 2:3], in0=sout[:, 0:1],
                                        scalar1=inv_n, scalar2=None,
                                        op0=AOP.mult)
                nc.vector.tensor_scalar(out=stat[:, 3:4], in0=sout[:, 1:2],
                                        scalar1=inv_n, scalar2=None,
                                        op0=AOP.mult)
                nc.vector.tensor_tensor(out=stat[:, 4:5], in0=stat[:, 2:3],
                                        in1=stat[:, 2:3], op=AOP.mult)
                nc.vector.tensor_tensor(out=stat[:, 4:5], in0=stat[:, 3:4],
                                        in1=stat[:, 4:5], op=AOP.subtract)
                nc.vector.tensor_scalar(out=stat[:, 4:5], in0=stat[:, 4:5],
                                        scalar1=float(EPS), scalar2=None,
                                        op0=AOP.add)
                nc.vector.reciprocal(stat[:, 5:6], stat[:, 4:5])
                nc.scalar.sqrt(stat[:, 6:7], stat[:, 5:6])
                nc.vector.tensor_tensor(out=stat[:, 6:7],
                                        in0=gb_sb[:, 2 * i:2 * i + 1],
                                        in1=stat[:, 6:7], op=AOP.mult)
                nc.vector.tensor_tensor(out=stat[:, 7:8], in0=stat[:, 6:7],
                                        in1=stat[:, 2:3], op=AOP.mult)
                nc.vector.tensor_tensor(out=stat[:, 7:8],
                                        in0=gb_sb[:, 2 * i + 1:2 * i + 2],
                                        in1=stat[:, 7:8], op=AOP.subtract)

                # y = func(scale*agg + shift); x = y or x + y (big fused ops)
                func = (mybir.ActivationFunctionType.Relu if i < L - 1
                        else mybir.ActivationFunctionType.Identity)
                if i == 0:
                    nc.scalar.activation(out=xT[:], in_=agg[:],
                                         func=func, bias=stat[:, 7:8],
                                         scale=stat[:, 6:7])
                else:
                    nc.scalar.activation(out=agg[:], in_=agg[:],
                                         func=func, bias=stat[:, 7:8],
                                         scale=stat[:, 6:7])
                    nc.vector.tensor_tensor(out=xT[:], in0=xT[:],
                                            in1=agg[:], op=AOP.add)

            nc.sync.dma_start(out=out_ext[:, :], in_=xT[:])
    nc.compile()
    return nc


# ------------------------------------------------------------------ runner

class _Runner:
    """Persistent-jit PJRT runner (run_bass_via_pjrt, callable repeatedly)."""

    def __init__(self, nc, n_cores):
        import jax
        from jax.experimental.shard_map import shard_map
        from jax.sharding import Mesh, PartitionSpec
        from concourse import bass2jax
        self.jax = jax
        bass2jax.install_neuronx_cc_hook()
        in_names, out_names, out_avals, zero_outs = [], [], [], []
        partition_name = (nc.partition_id_tensor.name
                          if nc.partition_id_tensor else None)
        for alloc in nc.m.functions[0].allocations:
            if not isinstance(alloc, mybir.MemoryLocationSet):
                continue
            name = alloc.memorylocations[0].name
            if alloc.kind == "ExternalInput":
                if name != partition_name:
                    in_names.append(name)
            elif alloc.kind == "ExternalOutput":
                out_names.append(name)
                shape = tuple(alloc.tensor_shape)
                dtype = mybir.dt.np(alloc.dtype)
                out_avals.append(jax.core.ShapedArray(shape, dtype))
                zero_outs.append(np.zeros(shape, dtype))
        self.in_names, self.out_names = in_names, out_names
        self.out_avals, self.zero_outs = out_avals, zero_outs
        n_params, n_outs = len(in_names), len(out_avals)
        all_in = list(in_names) + list(out_names)
        if partition_name is not None:
            all_in.append(partition_name)
        from concourse.bass2jax import _bass_exec_p, partition_id_tensor

        def _body(*args):
            operands = list(args)
            if partition_name is not None:
                operands.append(partition_id_tensor())
            outs = _bass_exec_p.bind(
                *operands, out_avals=tuple(out_avals),
                in_names=tuple(all_in), out_names=tuple(out_names),
                lowering_input_output_aliases=(),
                sim_require_finite=False, sim_require_nnan=False, nc=nc)
            return tuple(outs)

        devices = jax.devices()[:n_cores]
        self.n_cores = n_cores
        self.mesh = Mesh(np.asarray(devices), ("core",))
        in_specs = (PartitionSpec("core"),) * (n_params + n_outs)
        out_specs = (PartitionSpec("core"),) * len(out_names)
        self.fn = jax.jit(
            shard_map(_body, mesh=self.mesh, in_specs=in_specs,
                      out_specs=out_specs, check_rep=False),
            keep_unused=True)
        self.dev_in = None

    def put(self, in_maps):
        from jax.sharding import NamedSharding, PartitionSpec
        sh = NamedSharding(self.mesh, PartitionSpec("core"))
        n = self.n_cores
        concat_in = [
            np.concatenate([np.asarray(in_maps[c][name]) for c in range(n)],
                           axis=0)
            for name in self.in_names]
        concat_zeros = [np.zeros((n * z.shape[0], *z.shape[1:]), z.dtype)
                        for z in self.zero_outs]
        self.dev_in = [self.jax.device_put(a, sh)
                       for a in concat_in + concat_zeros]
        self.jax.block_until_ready(self.dev_in)

    def __call__(self, fetch=("out",)):
        out = self.fn(*self.dev_in)
        self.jax.block_until_ready(out)
        n = self.n_cores
        return [
            {name: np.asarray(out[i]).reshape(n, *self.out_avals[i].shape)[c]
             for i, name in enumerate(self.out_names) if name in fetch}
            for c in range(n)]


_CACHE = {}


def _build_from_prep(prep, N, D):
    n_real_last = prep["n_own"] - (prep["n_chunks"] - 1) * P
    return _build(D, prep["n_pad"], prep["n_chunks"], prep["NT"],
                  prep["tpc"], n_real_last, N)


def _get_runner(prep, N, D):
    key = (N, D, prep["NT"])
    if key in _CACHE:
        return _CACHE[key]
    nc = _build_from_prep(prep, N, D)
    r = _Runner(nc, CORES)
    _CACHE[key] = r
    return r


def _make_in_maps(prep, x, Ws, bs, gammas, betas):
    D = x.shape[1]
    Ws_flat = np.asarray(Ws, np.float32).reshape(L * D, D)
    gb = np.zeros((D, 2 * L), np.float32)
    for i in range(L):
        gb[:, 2 * i] = gammas[i]
        gb[:, 2 * i + 1] = betas[i]
    return [{"xT_in": prep["xT"][c], "src_in": prep["src"][c],
             "rel_in": prep["rel"][c], "disv_in": prep["disv"][c],
             "disc_in": prep["disc"][c], "dis2_in": prep["dis2b"][c],
             "Ws_in": Ws_flat, "gb_in": gb} for c in range(CORES)]


def _assemble_out(prep, res, N, D):
    n_own = prep["n_own"]
    out = np.empty((N, D), np.float32)
    for c in range(CORES):
        lo, hi = c * n_own, min((c + 1) * n_own, N)
        out[lo:hi] = res[c]["out"][:, :hi - lo].T
    return out


def kernel(x, edge_index, Ws, bs, gammas, betas):
    x = np.asarray(x, np.float32)
    edge_index = np.asarray(edge_index, np.int32)
    Ws = np.asarray(Ws, np.float32)
    gammas = np.asarray(gammas, np.float32)
    betas = np.asarray(betas, np.float32)
    N, D = x.shape

    prep = _prep(x, edge_index)
    r = _get_runner(prep, N, D)
    in_maps = _make_in_maps(prep, x, Ws, bs, gammas, betas)
    r.put(in_maps)
    res = r()
    return _assemble_out(prep, res, N, D)
